# revision 7
# baseline (speedup 1.0000x reference)
"""YOLOv3 detection-decode kernel for 8 Trainium2 NeuronCores.

Data-parallel over batch (16 images -> 2 per core). Per (scale, anchor) the
kernel processes BOTH local images in one macro-iteration:
  1. HWDGE-DMAs the x/y/conf/cls channels for both images into an f32
     staging tile xf [83, 2*HW] (rows: x,y then the 81 conf/cls channels);
     SWDGE cast-loads the raw w/h channels (f32->bf16) into rows 96:98 of a
     per-scale bf16 matmul-operand tile res [101, 2*HW+pad] whose rows
     83:96 are zeros and rows 98:101 hold resident [ones; gx; gy]
     constants (written once at startup).
  2. Per image, one scalar tanh(x/2) pass xf -> res[0:83] (sigmoid(x) =
     0.5*tanh(x/2) + 0.5; the ACT engine is column-serial, so splitting
     per image halves the op and unblocks image 0's matmuls early).
  3. Per image, ceil(HW/128) uniform 128-position chunks, one matmul each
     against a constant [101, 85] bf16 weight: transposes to [pos, 85],
     applies the 0.5/0.5 sigmoid affine + stride scaling, adds stride*grid
     offsets, and routes the raw w/h to columns 2/3. Chunk k covers
     positions {k + nchunk*i}; the last chunks of image 1 read junk from
     the pad columns, landing in PSUM partitions that are never stored.
  4. PSUM -> SBUF copies in 6-chunk batches cast to bf16; two tiny strided
     exp(x + ln anchor) ops per image fix up the w/h columns in place;
     SWDGE stores partitions [0:HW//nchunk] in one bulk DMA (nchunk*170B
     contiguous per partition) plus a single-descriptor tail. bf16 output
     halves store bytes; host upcasts to f32.
"""

import math
import os
import sys

import numpy as np

sys.path.insert(0, "/opt/trn_rl_repo")

N_CORES = 8
B_TOTAL = 16
B_LOC = B_TOTAL // N_CORES  # 2

INP_DIM = 608
NC_CLS = 80  # num classes
CH = 85  # 5 + classes
K_ROWS = 101  # 83 tanh'd + 13 zeros + 2 raw wh + ones + gx + gy
GI_ROWS = 16  # init block: 13 zero rows + ones + gx + gy

# (H, W, anchors[(w,h)x3]) per scale; strides 8/16/32
_SCALE_DEFS = [
    (76, 76, [(10.0, 13.0), (16.0, 30.0), (33.0, 23.0)]),
    (38, 38, [(30.0, 61.0), (62.0, 45.0), (59.0, 119.0)]),
    (19, 19, [(116.0, 90.0), (156.0, 198.0), (373.0, 326.0)]),
]


def _scales():
    out = []
    off = 0
    for h, w, anchors in _SCALE_DEFS:
        hw = h * w
        stride = INP_DIM // h
        nchunk = math.ceil(hw / 128)
        out.append(
            dict(
                H=h,
                W=w,
                HW=hw,
                stride=float(stride),
                anchors=anchors,
                off=off,
                nchunk=nchunk,
                padc=128 * nchunk - hw,
                pfull=hw // nchunk,
            )
        )
        off += 3 * hw
    return out, off


SCALES, N_ROWS = _scales()  # N_ROWS == 22743

# Smallest scale first (stores start flowing within ~2us) and smallest last
# (minimal store tail after the final load).
ITER_ORDER = [
    (2, 0),
    (0, 0), (0, 1), (0, 2),
    (1, 0), (1, 1), (1, 2),
    (2, 1), (2, 2),
]


def _make_weight(stride: float) -> np.ndarray:
    """[101, 85] matmul weight: transpose + sigmoid affine + grid/stride.
    All entries (0.5, 0.5*stride, stride, 1) are exact in bf16."""
    W = np.zeros((K_ROWS, CH), dtype=np.float32)
    W[0, 0] = W[1, 1] = 0.5 * stride  # x, y
    for p in range(2, 83):  # conf/cls: res row p holds channel p+2
        W[p, p + 2] = 0.5
    W[96, 2] = W[97, 3] = 1.0  # raw w, h pass through
    # ones row: sigmoid's +0.5 (stride-scaled for x/y)
    W[98, 0] = W[98, 1] = 0.5 * stride
    W[98, 4:] = 0.5
    W[99, 0] = stride  # gx row
    W[100, 1] = stride  # gy row
    return W


def _make_gridinit(h: int, w: int, padc: int) -> np.ndarray:
    """[16, B_LOC*HW + padc] init block for res rows 83:96 and 98:101:
    13 zero rows (their weight rows are zero, but NaN*0 would poison PSUM),
    then ones, grid_x, grid_y tiled per local image (zeros in the pad)."""
    hw2 = B_LOC * h * w
    gi = np.zeros((GI_ROWS, hw2 + padc), dtype=np.float32)
    go = np.empty((3, h * w), dtype=np.float32)
    go[0] = 1.0
    go[1] = np.tile(np.arange(w, dtype=np.float32), h)
    go[2] = np.repeat(np.arange(h, dtype=np.float32), w)
    gi[13:16, 0:hw2] = np.tile(go, (1, B_LOC))
    return gi


def _patch_tile_drain():
    """The kernel-tail drain Tile emits carries one sem-wait per outstanding
    processor; this container's walrus rejects >1 sync wait on a Drain
    (CoreV3 setupSyncWait "Too many sync wait commands"). Split the waits
    across a chain of single-wait drains — same semantics, compiles."""
    import concourse.mybir as mybir
    from concourse import tile as _tile
    from concourse.vector_clock import ScopedClock

    if getattr(_tile.TileContext, "_drain_split_patched", False):
        return

    def _drain_and_barrier(self, tick_clock, wait_clock):
        drain_inst = self.nc.sync.drain()
        wait_clock.add_sem_waits(
            drain_inst.ins, ScopedClock({None: tick_clock.global_clock})
        )
        si = drain_inst.ins.sync_info
        if si is not None and len(si.on_wait) > 1:
            extra = list(si.on_wait[1:])
            del si.on_wait[1:]
            for w in extra:
                d2 = self.nc.sync.drain()
                si2 = d2.ins.sync_info
                if si2 is None:
                    d2.ins.sync_info = mybir.SyncInfo(on_wait=[w], on_update=[])
                else:
                    si2.on_wait.append(w)
        self.nc.all_engine_barrier()
        assert self.sems is not None
        popped = self.nc._tile_sem_poison_stack.pop()
        assert popped is self._sem_poison
        self.nc.clear_and_free_semaphores(list(self.sems.allocated().values()))
        self.nc.all_engine_barrier()

    _tile.TileContext._drain_and_barrier = _drain_and_barrier
    _tile.TileContext._drain_split_patched = True


_WAIT_CAP = 1


def _split_sync_waits(bir_json: bytes) -> bytes:
    """This container's walrus rejects instructions carrying more than one
    sync wait command. Move extra waits onto injected NoOps immediately
    before the instruction on the same engine queue (sequencers execute in
    order, so the combined wait semantics are identical)."""
    import json as _json

    d = _json.loads(bir_json)
    n = 0
    for f in d.get("functions", []):
        for bb in f.get("blocks", []):
            ins_list = bb.get("instructions", [])
            out = []
            for ins in ins_list:
                si = ins.get("sync_info")
                waits = (si or {}).get("on_wait") or []
                if len(waits) > _WAIT_CAP:
                    keep = waits[-_WAIT_CAP:]
                    extra = waits[: -_WAIT_CAP]
                    for i in range(0, len(extra), _WAIT_CAP):
                        n += 1
                        out.append(
                            {
                                "name": f"I-wsplit-{n}",
                                "opcode": "NoOp",
                                "engine": ins["engine"],
                                "ins": [],
                                "outs": [],
                                "bass_nofuse": True,
                                "sync_info": {
                                    "on_wait": extra[i : i + _WAIT_CAP],
                                    "on_update": [],
                                },
                            }
                        )
                    si["on_wait"] = keep
                out.append(ins)
            bb["instructions"] = out
    return _json.dumps(d).encode()


def _patch_compile():
    import concourse.bass_utils as bu

    if getattr(bu, "_wait_split_patched", False):
        return
    orig = bu.compile_bir_kernel

    def compile_bir_kernel_split(bir_json, tmpdir, neff_name="file.neff"):
        return orig(_split_sync_waits(bir_json), tmpdir, neff_name)

    bu.compile_bir_kernel = compile_bir_kernel_split
    bu._wait_split_patched = True
    import concourse.bass2jax as b2j

    b2j.compile_bir_kernel = compile_bir_kernel_split


def _build_program():
    import concourse.bass as bass
    import concourse.mybir as mybir
    from concourse.tile import TileContext

    _patch_tile_drain()
    _patch_compile()

    f32 = mybir.dt.float32
    bf16 = mybir.dt.bfloat16
    AF = mybir.ActivationFunctionType

    nc = bass.Bass()

    x_dram = [
        nc.dram_tensor(f"x{s}", [B_LOC, 255, sc["HW"]], f32, kind="ExternalInput")
        for s, sc in enumerate(SCALES)
    ]
    w_dram = [
        nc.dram_tensor(f"w{s}", [K_ROWS, CH], bf16, kind="ExternalInput")
        for s in range(3)
    ]
    gi_dram = [
        nc.dram_tensor(
            f"gi{s}",
            [GI_ROWS, B_LOC * SCALES[s]["HW"] + SCALES[s]["padc"]],
            bf16,
            kind="ExternalInput",
        )
        for s in range(3)
    ]
    # ln(anchor) activation biases, one column per (scale, anchor, w|h),
    # replicated down all 128 partitions.
    ln_dram = nc.dram_tensor("lnA", [128, 18], f32, kind="ExternalInput")
    out = nc.dram_tensor("out", [B_LOC, N_ROWS, CH], bf16, kind="ExternalOutput")

    GROUP = 6  # transpose chunks per PSUM bank (6*85 = 510 <= 512 f32)

    with TileContext(nc) as tc:
        with (
            tc.tile_pool(name="consts", bufs=1) as cpool,
            tc.tile_pool(name="stage", bufs=2) as spool,
            tc.tile_pool(name="obuf", bufs=4) as opool,
            tc.tile_pool(name="psum", bufs=6, space="PSUM") as ppool,
        ):
            # Dependency-free dummy activation: forces the shared tanh/exp
            # ACT table load at stream start instead of behind the first
            # iteration's load-wait.
            scratch = cpool.tile([1, 8], f32, tag="scratch")
            nc.scalar.activation(
                out=scratch[0:1, 0:1], in_=scratch[0:1, 0:1], func=AF.Tanh
            )

            # Allocate const tiles up front; DMA them lazily, interleaved
            # into the sync stream right after the loads of the iteration
            # that first needs them (so iteration-0's input loads issue
            # first and the pipeline primes immediately).
            w_sb = [cpool.tile([K_ROWS, CH], bf16, tag=f"w{s}", name=f"w{s}") for s in range(3)]
            ln_sb = cpool.tile([128, 18], f32, tag="lnA")
            res_t = [
                [
                    cpool.tile(
                        [K_ROWS, B_LOC * sc["HW"] + sc["padc"]],
                        bf16,
                        tag=f"res{s}_{bi}",
                        name=f"res{s}_{bi}",
                    )
                    for bi in range(2)
                ]
                for s, sc in enumerate(SCALES)
            ]
            w_loaded = [False] * 3
            gi_loaded = [[False, False] for _ in range(3)]
            ln_loaded = False

            scale_ctr = [0, 0, 0]
            for s, a in ITER_ORDER:
                sc = SCALES[s]
                hw = sc["HW"]
                hw2 = B_LOC * hw
                nchunk = sc["nchunk"]
                pfull = sc["pfull"]
                tail = hw - pfull * nchunk
                bi = scale_ctr[s] % 2
                res = res_t[s][bi]
                scale_ctr[s] += 1
                c0 = 85 * a

                # f32 staging loads for the tanh'd channels (HWDGE spreads a
                # P-partition load over the largest divisor of P <= 16
                # engines, so split 2/80/1).
                xf = spool.tile([83, hw2], f32, tag="xf")
                nc.sync.dma_start(
                    out=xf[0:2, :],
                    in_=x_dram[s][:, c0 : c0 + 2, :].rearrange("b p w -> p b w"),
                )
                nc.sync.dma_start(
                    out=xf[2:82, :],
                    in_=x_dram[s][:, c0 + 4 : c0 + 84, :].rearrange("b p w -> p b w"),
                )
                nc.sync.dma_start(
                    out=xf[82:83, :],
                    in_=x_dram[s][:, c0 + 84 : c0 + 85, :].rearrange("b p w -> p b w"),
                )
                # raw w/h rows, cast f32->bf16 during DMA (SWDGE)
                nc.gpsimd.dma_start(
                    out=res[96:98, :hw2],
                    in_=x_dram[s][:, c0 + 2 : c0 + 4, :].rearrange("b p w -> p b w"),
                )

                # Lazy one-time const loads (after this iteration's input
                # loads so they never delay pipeline priming).
                if not gi_loaded[s][bi]:
                    nc.sync.dma_start(out=res[83:96, :], in_=gi_dram[s][0:13, :])
                    nc.sync.dma_start(out=res[98:101, :], in_=gi_dram[s][13:16, :])
                    gi_loaded[s][bi] = True
                if not w_loaded[s]:
                    nc.sync.dma_start(out=w_sb[s][:], in_=w_dram[s][:])
                    w_loaded[s] = True
                if not ln_loaded:
                    nc.sync.dma_start(out=ln_sb[:], in_=ln_dram[:])
                    ln_loaded = True

                base = sc["off"] + a * hw
                for b in range(B_LOC):
                    cb = b * hw
                    # sigmoid via tanh; tanh and exp share one ACT table set.
                    nc.scalar.activation(
                        out=res[0:83, cb : cb + hw],
                        in_=xf[0:83, cb : cb + hw],
                        func=AF.Tanh,
                        scale=0.5,
                    )
                    obuf = opool.tile([128, nchunk * CH], bf16, tag="obuf")
                    # Strided position chunks: chunk k covers positions
                    # {k + nchunk*i}, so PSUM/obuf partition i accumulates
                    # nchunk consecutive output rows -> the store DMA gets
                    # nchunk*170B contiguous per partition.
                    res_str = res[:, cb : cb + 128 * nchunk].rearrange(
                        "p (i r) -> p r i", r=nchunk
                    )
                    ngroups = math.ceil(nchunk / GROUP)
                    for g in range(ngroups):
                        k0 = g * GROUP
                        k1 = min(k0 + GROUP, nchunk)
                        psum = ppool.tile([128, GROUP * CH], f32, tag="ps")
                        for k in range(k0, k1):
                            nc.tensor.matmul(
                                psum[:, (k - k0) * CH : (k - k0) * CH + CH],
                                lhsT=res_str[:, k, :],
                                rhs=w_sb[s][:],
                                start=True,
                                stop=True,
                            )
                        wcols = (k1 - k0) * CH
                        nc.vector.tensor_copy(
                            out=obuf[:, k0 * CH : k0 * CH + wcols],
                            in_=psum[:, 0:wcols],
                        )

                    # w/h: exp(x + ln anchor) in place on strided cols 2,3.
                    # Junk partitions (never stored) may blow up harmlessly.
                    ob3 = obuf.rearrange("p (k c) -> p k c", c=CH)
                    for col in (2, 3):
                        j = (s * 3 + a) * 2 + (col - 2)
                        nc.scalar.activation(
                            out=ob3[:, 0:nchunk, col : col + 1],
                            in_=ob3[:, 0:nchunk, col : col + 1],
                            func=AF.Exp,
                            bias=ln_sb[:, j : j + 1],
                        )

                    # partition p <-> rows [base + p*nchunk, +nchunk): one
                    # contiguous nchunk*170B descriptor per partition, plus
                    # a single-descriptor tail from partition pfull.
                    dst = out[b, base : base + pfull * nchunk, :].rearrange(
                        "(p r) c -> p (r c)", p=pfull
                    )
                    nc.gpsimd.dma_start(out=dst, in_=obuf[0:pfull, 0 : nchunk * CH])
                    if tail:
                        nc.gpsimd.dma_start(
                            out=out[b, base + pfull * nchunk : base + hw, :],
                            in_=obuf[pfull : pfull + 1, 0 : tail * CH],
                        )
    return nc


_PROGRAM = None
LAST_RESULT = None


def _get_program():
    global _PROGRAM
    if _PROGRAM is None:
        _PROGRAM = _build_program()
    return _PROGRAM


def kernel(x1: np.ndarray, x2: np.ndarray, x3: np.ndarray) -> np.ndarray:
    global LAST_RESULT
    from concourse.bass_utils import run_bass_kernel_spmd

    nc = _get_program()

    import ml_dtypes

    bf16 = ml_dtypes.bfloat16
    xs = [
        np.ascontiguousarray(x, dtype=np.float32).reshape(B_TOTAL, 255, sc["HW"])
        for x, sc in zip((x1, x2, x3), SCALES)
    ]
    w_consts = [_make_weight(sc["stride"]).astype(bf16) for sc in SCALES]
    gi_consts = [
        _make_gridinit(sc["H"], sc["W"], sc["padc"]).astype(bf16) for sc in SCALES
    ]
    ln_vals = np.array(
        [math.log(v) for sc in SCALES for anc in sc["anchors"] for v in anc],
        dtype=np.float32,
    )
    ln_const = np.broadcast_to(ln_vals, (128, 18)).copy()

    in_maps = []
    for i in range(N_CORES):
        m = {"lnA": ln_const}
        for s in range(3):
            m[f"x{s}"] = xs[s][i * B_LOC : (i + 1) * B_LOC]
            m[f"w{s}"] = w_consts[s]
            m[f"gi{s}"] = gi_consts[s]
        in_maps.append(m)

    LAST_RESULT = run_bass_kernel_spmd(nc, in_maps, core_ids=list(range(N_CORES)))
    return np.concatenate([r["out"] for r in LAST_RESULT.results], axis=0).astype(
        np.float32
    )


# revision 9
# speedup vs baseline: 1.0688x; 1.0688x over previous
"""YOLOv3 detection-decode kernel for 8 Trainium2 NeuronCores.

Data-parallel over batch (16 images -> 2 per core). Per (scale, anchor) the
kernel processes BOTH local images in one macro-iteration:
  1. HWDGE-DMAs the x/y/conf/cls channels for both images into an f32
     staging tile xf [83, 2*HW] (rows: x,y then the 81 conf/cls channels);
     SWDGE cast-loads the raw w/h channels (f32->bf16) into rows 96:98 of a
     per-scale bf16 matmul-operand tile res [101, 2*HW+pad] whose rows
     83:96 are zeros and rows 98:101 hold resident [ones; gx; gy]
     constants (written once at startup).
  2. Per image, one scalar tanh(x/2) pass xf -> res[0:83] (sigmoid(x) =
     0.5*tanh(x/2) + 0.5; the ACT engine is column-serial, so splitting
     per image halves the op and unblocks image 0's matmuls early).
  3. Per image, ceil(HW/128) uniform 128-position chunks, one matmul each
     against a constant [101, 85] bf16 weight: transposes to [pos, 85],
     applies the 0.5/0.5 sigmoid affine + stride scaling, adds stride*grid
     offsets, and routes the raw w/h to columns 2/3. Chunk k covers
     positions {k + nchunk*i}; the last chunks of image 1 read junk from
     the pad columns, landing in PSUM partitions that are never stored.
  4. PSUM -> SBUF copies in 6-chunk batches cast to bf16; two tiny strided
     exp(x + ln anchor) ops per image fix up the w/h columns in place;
     SWDGE stores partitions [0:HW//nchunk] in one bulk DMA (nchunk*170B
     contiguous per partition) plus a single-descriptor tail. bf16 output
     halves store bytes; host upcasts to f32.
"""

import math
import os
import sys

import numpy as np

sys.path.insert(0, "/opt/trn_rl_repo")

N_CORES = 8
B_TOTAL = 16
B_LOC = B_TOTAL // N_CORES  # 2

INP_DIM = 608
NC_CLS = 80  # num classes
CH = 85  # 5 + classes
K_ROWS = 101  # 83 tanh'd + 13 zeros + 2 raw wh + ones + gx + gy
GI_ROWS = 16  # init block: 13 zero rows + ones + gx + gy

# (H, W, anchors[(w,h)x3]) per scale; strides 8/16/32
_SCALE_DEFS = [
    (76, 76, [(10.0, 13.0), (16.0, 30.0), (33.0, 23.0)]),
    (38, 38, [(30.0, 61.0), (62.0, 45.0), (59.0, 119.0)]),
    (19, 19, [(116.0, 90.0), (156.0, 198.0), (373.0, 326.0)]),
]


def _scales():
    out = []
    off = 0
    for h, w, anchors in _SCALE_DEFS:
        hw = h * w
        stride = INP_DIM // h
        nchunk = math.ceil(hw / 128)
        out.append(
            dict(
                H=h,
                W=w,
                HW=hw,
                stride=float(stride),
                anchors=anchors,
                off=off,
                nchunk=nchunk,
                padc=128 * nchunk - hw,
                pfull=hw // nchunk,
            )
        )
        off += 3 * hw
    return out, off


SCALES, N_ROWS = _scales()  # N_ROWS == 22743

# Smallest scale first (stores start flowing within ~2us) and smallest last
# (minimal store tail after the final load).
ITER_ORDER = [
    (2, 0),
    (0, 0), (0, 1), (0, 2),
    (1, 0), (1, 1), (1, 2),
    (2, 1), (2, 2),
]


def _make_weight(stride: float) -> np.ndarray:
    """[101, 85] matmul weight: transpose + sigmoid affine + grid/stride.
    All entries (0.5, 0.5*stride, stride, 1) are exact in bf16."""
    W = np.zeros((K_ROWS, CH), dtype=np.float32)
    W[0, 0] = W[1, 1] = 0.5 * stride  # x, y
    for p in range(2, 83):  # conf/cls: res row p holds channel p+2
        W[p, p + 2] = 0.5
    W[96, 2] = W[97, 3] = 1.0  # raw w, h pass through
    # ones row: sigmoid's +0.5 (stride-scaled for x/y)
    W[98, 0] = W[98, 1] = 0.5 * stride
    W[98, 4:] = 0.5
    W[99, 0] = stride  # gx row
    W[100, 1] = stride  # gy row
    return W


def _make_gridinit(h: int, w: int, padc: int) -> np.ndarray:
    """[16, B_LOC*HW + padc] init block for res rows 83:96 and 98:101:
    13 zero rows (their weight rows are zero, but NaN*0 would poison PSUM),
    then ones, grid_x, grid_y tiled per local image (zeros in the pad)."""
    hw2 = B_LOC * h * w
    gi = np.zeros((GI_ROWS, hw2 + padc), dtype=np.float32)
    go = np.empty((3, h * w), dtype=np.float32)
    go[0] = 1.0
    go[1] = np.tile(np.arange(w, dtype=np.float32), h)
    go[2] = np.repeat(np.arange(h, dtype=np.float32), w)
    gi[13:16, 0:hw2] = np.tile(go, (1, B_LOC))
    return gi


def _patch_tile_drain():
    """The kernel-tail drain Tile emits carries one sem-wait per outstanding
    processor; this container's walrus rejects >1 sync wait on a Drain
    (CoreV3 setupSyncWait "Too many sync wait commands"). Split the waits
    across a chain of single-wait drains — same semantics, compiles."""
    import concourse.mybir as mybir
    from concourse import tile as _tile
    from concourse.vector_clock import ScopedClock

    if getattr(_tile.TileContext, "_drain_split_patched", False):
        return

    def _drain_and_barrier(self, tick_clock, wait_clock):
        drain_inst = self.nc.sync.drain()
        wait_clock.add_sem_waits(
            drain_inst.ins, ScopedClock({None: tick_clock.global_clock})
        )
        si = drain_inst.ins.sync_info
        if si is not None and len(si.on_wait) > 1:
            extra = list(si.on_wait[1:])
            del si.on_wait[1:]
            for w in extra:
                d2 = self.nc.sync.drain()
                si2 = d2.ins.sync_info
                if si2 is None:
                    d2.ins.sync_info = mybir.SyncInfo(on_wait=[w], on_update=[])
                else:
                    si2.on_wait.append(w)
        self.nc.all_engine_barrier()
        assert self.sems is not None
        popped = self.nc._tile_sem_poison_stack.pop()
        assert popped is self._sem_poison
        self.nc.clear_and_free_semaphores(list(self.sems.allocated().values()))
        self.nc.all_engine_barrier()

    _tile.TileContext._drain_and_barrier = _drain_and_barrier
    _tile.TileContext._drain_split_patched = True


_WAIT_CAP = 1


def _split_sync_waits(bir_json: bytes) -> bytes:
    """This container's walrus rejects instructions carrying more than one
    sync wait command. Move extra waits onto injected NoOps immediately
    before the instruction on the same engine queue (sequencers execute in
    order, so the combined wait semantics are identical)."""
    import json as _json

    d = _json.loads(bir_json)
    n = 0
    for f in d.get("functions", []):
        for bb in f.get("blocks", []):
            ins_list = bb.get("instructions", [])
            out = []
            for ins in ins_list:
                si = ins.get("sync_info")
                waits = (si or {}).get("on_wait") or []
                if len(waits) > _WAIT_CAP:
                    keep = waits[-_WAIT_CAP:]
                    extra = waits[: -_WAIT_CAP]
                    for i in range(0, len(extra), _WAIT_CAP):
                        n += 1
                        out.append(
                            {
                                "name": f"I-wsplit-{n}",
                                "opcode": "NoOp",
                                "engine": ins["engine"],
                                "ins": [],
                                "outs": [],
                                "bass_nofuse": True,
                                "sync_info": {
                                    "on_wait": extra[i : i + _WAIT_CAP],
                                    "on_update": [],
                                },
                            }
                        )
                    si["on_wait"] = keep
                out.append(ins)
            bb["instructions"] = out
    return _json.dumps(d).encode()


def _patch_compile():
    import concourse.bass_utils as bu

    if getattr(bu, "_wait_split_patched", False):
        return
    orig = bu.compile_bir_kernel

    def compile_bir_kernel_split(bir_json, tmpdir, neff_name="file.neff"):
        return orig(_split_sync_waits(bir_json), tmpdir, neff_name)

    bu.compile_bir_kernel = compile_bir_kernel_split
    bu._wait_split_patched = True
    import concourse.bass2jax as b2j

    b2j.compile_bir_kernel = compile_bir_kernel_split


def _build_program():
    import concourse.bass as bass
    import concourse.mybir as mybir
    from concourse.tile import TileContext

    _patch_tile_drain()
    _patch_compile()

    f32 = mybir.dt.float32
    bf16 = mybir.dt.bfloat16
    AF = mybir.ActivationFunctionType

    nc = bass.Bass()

    x_dram = [
        nc.dram_tensor(f"x{s}", [B_LOC, 255, sc["HW"]], f32, kind="ExternalInput")
        for s, sc in enumerate(SCALES)
    ]
    w_dram = [
        nc.dram_tensor(f"w{s}", [K_ROWS, CH], bf16, kind="ExternalInput")
        for s in range(3)
    ]
    gi_dram = [
        nc.dram_tensor(
            f"gi{s}",
            [GI_ROWS, B_LOC * SCALES[s]["HW"] + SCALES[s]["padc"]],
            bf16,
            kind="ExternalInput",
        )
        for s in range(3)
    ]
    # ln(anchor) activation biases, one column per (scale, anchor, w|h),
    # replicated down all 128 partitions.
    ln_dram = nc.dram_tensor("lnA", [128, 18], f32, kind="ExternalInput")
    out = nc.dram_tensor("out", [B_LOC, N_ROWS, CH], bf16, kind="ExternalOutput")

    GROUP = 6  # transpose chunks per PSUM bank (6*85 = 510 <= 512 f32)

    with TileContext(nc) as tc:
        with (
            tc.tile_pool(name="consts", bufs=1) as cpool,
            tc.tile_pool(name="obuf", bufs=8) as opool,
            tc.tile_pool(name="psum", bufs=6, space="PSUM") as ppool,
        ):
            # Dependency-free dummy activation: forces the shared tanh/exp
            # ACT table load at stream start instead of behind the first
            # iteration's load-wait.
            scratch = cpool.tile([1, 8], f32, tag="scratch")
            nc.scalar.activation(
                out=scratch[0:1, 0:1], in_=scratch[0:1, 0:1], func=AF.Tanh
            )

            # Allocate const tiles up front; DMA them lazily, interleaved
            # into the sync stream right after the loads of the iteration
            # that first needs them (so iteration-0's input loads issue
            # first and the pipeline primes immediately).
            w_sb = [cpool.tile([K_ROWS, CH], bf16, tag=f"w{s}", name=f"w{s}") for s in range(3)]
            ln_sb = cpool.tile([128, 18], f32, tag="lnA")
            res_t = [
                [
                    cpool.tile(
                        [K_ROWS, B_LOC * sc["HW"] + sc["padc"]],
                        bf16,
                        tag=f"res{s}_{bi}",
                        name=f"res{s}_{bi}",
                    )
                    for bi in range(2)
                ]
                for s, sc in enumerate(SCALES)
            ]
            w_loaded = [False] * 3
            gi_loaded = [[False, False] for _ in range(3)]
            ln_loaded = False

            pending = []

            def _flush(item):
                fs, fa, fb, fobuf = item
                fsc = SCALES[fs]
                fnchunk = fsc["nchunk"]
                fpfull = fsc["pfull"]
                ftail = fsc["HW"] - fpfull * fnchunk
                fbase = fsc["off"] + fa * fsc["HW"]
                # w/h: exp(x + ln anchor) in place on strided cols 2,3.
                # Junk partitions (never stored) may blow up harmlessly.
                ob3 = fobuf.rearrange("p (k c) -> p k c", c=CH)
                for col in (2, 3):
                    j = (fs * 3 + fa) * 2 + (col - 2)
                    nc.scalar.activation(
                        out=ob3[:, 0:fnchunk, col : col + 1],
                        in_=ob3[:, 0:fnchunk, col : col + 1],
                        func=AF.Exp,
                        bias=ln_sb[:, j : j + 1],
                    )
                # partition p <-> rows [fbase + p*fnchunk, +fnchunk): one
                # contiguous fnchunk*170B descriptor per partition, plus
                # a single-descriptor tail from partition fpfull.
                dst = out[
                    fb, fbase : fbase + fpfull * fnchunk, :
                ].rearrange("(p r) c -> p (r c)", p=fpfull)
                nc.sync.dma_start(out=dst, in_=fobuf[0:fpfull, 0 : fnchunk * CH])
                if ftail:
                    nc.gpsimd.dma_start(
                        out=out[fb, fbase + fpfull * fnchunk : fbase + fsc["HW"], :],
                        in_=fobuf[fpfull : fpfull + 1, 0 : ftail * CH],
                    )

            scale_ctr = [0, 0, 0]
            for s, a in ITER_ORDER:
                sc = SCALES[s]
                hw = sc["HW"]
                hw2 = B_LOC * hw
                nchunk = sc["nchunk"]
                pfull = sc["pfull"]
                tail = hw - pfull * nchunk
                bi = scale_ctr[s] % 2
                res = res_t[s][bi]
                scale_ctr[s] += 1
                c0 = 85 * a

                # Cast-load (f32->bf16, SWDGE) straight into the matmul
                # tile: x/y rows, conf/cls rows, then raw w/h rows.
                nc.gpsimd.dma_start(
                    out=res[0:2, :hw2],
                    in_=x_dram[s][:, c0 : c0 + 2, :].rearrange("b p w -> p b w"),
                )
                nc.gpsimd.dma_start(
                    out=res[2:83, :hw2],
                    in_=x_dram[s][:, c0 + 4 : c0 + 85, :].rearrange("b p w -> p b w"),
                )
                nc.gpsimd.dma_start(
                    out=res[96:98, :hw2],
                    in_=x_dram[s][:, c0 + 2 : c0 + 4, :].rearrange("b p w -> p b w"),
                )

                # Lazy one-time const loads (after this iteration's input
                # loads so they never delay pipeline priming).
                if not gi_loaded[s][bi]:
                    nc.sync.dma_start(out=res[83:96, :], in_=gi_dram[s][0:13, :])
                    nc.sync.dma_start(out=res[98:101, :], in_=gi_dram[s][13:16, :])
                    gi_loaded[s][bi] = True
                if not w_loaded[s]:
                    nc.sync.dma_start(out=w_sb[s][:], in_=w_dram[s][:])
                    w_loaded[s] = True
                if not ln_loaded:
                    nc.sync.dma_start(out=ln_sb[:], in_=ln_dram[:])
                    ln_loaded = True

                base = sc["off"] + a * hw
                for b in range(B_LOC):
                    cb = b * hw
                    # sigmoid via tanh; tanh and exp share one ACT table set.
                    nc.scalar.activation(
                        out=res[0:83, cb : cb + hw],
                        in_=res[0:83, cb : cb + hw],
                        func=AF.Tanh,
                        scale=0.5,
                    )
                    obuf = opool.tile([128, nchunk * CH], bf16, tag="obuf")
                    # Strided position chunks: chunk k covers positions
                    # {k + nchunk*i}, so PSUM/obuf partition i accumulates
                    # nchunk consecutive output rows -> the store DMA gets
                    # nchunk*170B contiguous per partition.
                    res_str = res[:, cb : cb + 128 * nchunk].rearrange(
                        "p (i r) -> p r i", r=nchunk
                    )
                    ngroups = math.ceil(nchunk / GROUP)
                    for g in range(ngroups):
                        k0 = g * GROUP
                        k1 = min(k0 + GROUP, nchunk)
                        psum = ppool.tile([128, GROUP * CH], f32, tag="ps")
                        for k in range(k0, k1):
                            nc.tensor.matmul(
                                psum[:, (k - k0) * CH : (k - k0) * CH + CH],
                                lhsT=res_str[:, k, :],
                                rhs=w_sb[s][:],
                                start=True,
                                stop=True,
                            )
                        wcols = (k1 - k0) * CH
                        nc.vector.tensor_copy(
                            out=obuf[:, k0 * CH : k0 * CH + wcols],
                            in_=psum[:, 0:wcols],
                        )
                    pending.append((s, a, b, obuf))

                # Software pipelining: emit the previous iteration's exps +
                # stores AFTER this iteration's tanh/matmuls, so the scalar
                # sequencer's in-order stream never stalls the next
                # iteration's tanh behind exp -> copy -> matmul deps.
                while len(pending) > 2:
                    _flush(pending.pop(0))
            while pending:
                _flush(pending.pop(0))
    return nc


_PROGRAM = None
LAST_RESULT = None


def _get_program():
    global _PROGRAM
    if _PROGRAM is None:
        _PROGRAM = _build_program()
    return _PROGRAM


def kernel(x1: np.ndarray, x2: np.ndarray, x3: np.ndarray) -> np.ndarray:
    global LAST_RESULT
    from concourse.bass_utils import run_bass_kernel_spmd

    nc = _get_program()

    import ml_dtypes

    bf16 = ml_dtypes.bfloat16
    xs = [
        np.ascontiguousarray(x, dtype=np.float32).reshape(B_TOTAL, 255, sc["HW"])
        for x, sc in zip((x1, x2, x3), SCALES)
    ]
    w_consts = [_make_weight(sc["stride"]).astype(bf16) for sc in SCALES]
    gi_consts = [
        _make_gridinit(sc["H"], sc["W"], sc["padc"]).astype(bf16) for sc in SCALES
    ]
    ln_vals = np.array(
        [math.log(v) for sc in SCALES for anc in sc["anchors"] for v in anc],
        dtype=np.float32,
    )
    ln_const = np.broadcast_to(ln_vals, (128, 18)).copy()

    in_maps = []
    for i in range(N_CORES):
        m = {"lnA": ln_const}
        for s in range(3):
            m[f"x{s}"] = xs[s][i * B_LOC : (i + 1) * B_LOC]
            m[f"w{s}"] = w_consts[s]
            m[f"gi{s}"] = gi_consts[s]
        in_maps.append(m)

    LAST_RESULT = run_bass_kernel_spmd(nc, in_maps, core_ids=list(range(N_CORES)))
    return np.concatenate([r["out"] for r in LAST_RESULT.results], axis=0).astype(
        np.float32
    )
